# revision 6
# baseline (speedup 1.0000x reference)
"""BiLSTM-CRF Trainium2 kernel (8 NeuronCores, single launch).

Strategy
--------
The 7-layer BiLSTM recurrence is sequence-parallelized: LSTM state from a
wrong (zero) initial state converges to the true state in ~32 steps (forget
gates ~0.5 here), so each direction's 8192-step scan is split into 256
chunks of 32 steps, each preceded by a 32-step warmup.  4 cores process the
fwd direction, 4 the bwd direction (as data: bwd cores see the reversed
sequence and their own direction's weights; the program is SPMD-uniform).
Each core runs 64 lockstep "streams" (chunks); the per-step hidden matvec
amortizes the PE weight loads across all 64 streams.

Between layers, cores exchange hidden states with an AllGather through HBM.
The final FC + Viterbi forward pass (backpointers) also run on device
(Viterbi is window-parallel: backpointers coalesce in <32 steps); the
backtrace + score run on the host (tiny).
"""

import os
import numpy as np

import concourse.bass as bass
import concourse.mybir as mybir
import concourse.tile as tile
from concourse import bacc
from concourse.bass_utils import run_bass_kernel_spmd

F32 = mybir.dt.float32
BF16 = mybir.dt.bfloat16
U32 = mybir.dt.uint32
AF = mybir.ActivationFunctionType
ALU = mybir.AluOpType

# problem constants
T = 8192
NH = 256
NTAG = 7
START_TAG = 5
STOP_TAG = 6
NLAYERS = 7

# sharding config
NCORE = 8
SPAN = T // 4          # 2048 positions per core (per direction)
S = 64                 # streams (chunks) per core
LEN = SPAN // S        # 32 positions per stream
W = 32                 # LSTM warmup steps
STEPS = LEN + W        # recurrence steps per layer
L = SPAN + W           # 2080 = per-core x/xp position domain

# viterbi config
VWIN = 128             # windows per core
VLEN = (T // NCORE) // VWIN   # 8 positions per window
VW = 32                # viterbi warmup steps
VSTEPS = VLEN + VW     # 40
BIG = 1.0e6
NEG = -10000.0

# gate permutation: torch order [i f g o] -> ours [i0 i1 f0 f1 o0 o1 g0 g1]
GPERM = np.concatenate([
    np.arange(0, 256),        # i
    np.arange(256, 512),      # f
    np.arange(768, 1024),     # o
    np.arange(512, 768),      # g
])

XP_DT = F32 if os.environ.get("K_XP_F32") else BF16


def rap(ap_or_handle, extra_off, pattern):
    """Raw access-pattern view. ap_or_handle: AP (from tile[:]) or dram handle."""
    if isinstance(ap_or_handle, bass.AP):
        base = ap_or_handle
        off = base.offset + extra_off if isinstance(base.offset, int) else None
        assert off is not None
        return bass.AP(base.tensor, off, pattern)
    return bass.AP(ap_or_handle, extra_off, pattern)


# ---------------------------------------------------------------------------
# device program
# ---------------------------------------------------------------------------

def build_program():
    nc = bacc.Bacc("TRN2", target_bir_lowering=False, debug=False,
                   num_devices=NCORE)
    dbg_layer = int(os.environ.get("KDBG_LAYER", "-1"))

    # ---- external inputs (per core) ----
    xt0 = nc.dram_tensor("xt0", [3, L], F32, kind="ExternalInput")
    wih0 = nc.dram_tensor("wih0", [3, 1024], F32, kind="ExternalInput")
    whh = [nc.dram_tensor(f"whh{l}", [128, 2048], F32, kind="ExternalInput")
           for l in range(NLAYERS)]
    wih = [None] + [nc.dram_tensor(f"wih{l}", [128, 4096], F32, kind="ExternalInput")
                    for l in range(1, NLAYERS)]
    bia = [nc.dram_tensor(f"bias{l}", [128, 8], F32, kind="ExternalInput")
           for l in range(NLAYERS)]
    fcw_in = nc.dram_tensor("fcw", [128, 4 * NTAG], F32, kind="ExternalInput")
    fcb_in = nc.dram_tensor("fcb", [128, NTAG], F32, kind="ExternalInput")
    trans_in = nc.dram_tensor("transB", [128, 49], F32, kind="ExternalInput")
    iot_in = nc.dram_tensor("iotB", [128, 49], F32, kind="ExternalInput")
    xpadd_in = nc.dram_tensor("xpadd", [128, 8 * W], F32, kind="ExternalInput")
    xpsc_in = nc.dram_tensor("xpsc", [128, 1], F32, kind="ExternalInput")
    vmask_in = nc.dram_tensor("vmask", [128, VSTEPS * NTAG], F32, kind="ExternalInput")
    vadd_in = nc.dram_tensor("vadd", [128, VSTEPS * NTAG], F32, kind="ExternalInput")
    prm = nc.dram_tensor("prm", [1, 16], U32, kind="ExternalInput")

    # ---- external outputs (per core) ----
    bpt_out = nc.dram_tensor("bpt", [128, VLEN * NTAG], F32, kind="ExternalOutput")
    fvo_out = nc.dram_tensor("fvo", [1, NTAG], F32, kind="ExternalOutput")
    fco_out = nc.dram_tensor("fco", [T // NCORE, NTAG], F32, kind="ExternalOutput")
    if dbg_layer >= 0:
        dbgc_out = nc.dram_tensor("dbgc", [NH, SPAN], F32, kind="ExternalOutput")
        dbgx_out = nc.dram_tensor("dbgx", [128, 4 * L], F32, kind="ExternalOutput")

    rg = [list(range(NCORE))]

    with tile.TileContext(nc) as tc:
        with (
            tc.tile_pool(name="const", bufs=1) as cpool,
            tc.tile_pool(name="wpool", bufs=1) as wpool,
            tc.tile_pool(name="xppool", bufs=1) as xppool,
            tc.tile_pool(name="xbuf", bufs=2) as xpool,
            tc.tile_pool(name="state", bufs=1) as spool,
            tc.tile_pool(name="work", bufs=2) as work,
            tc.tile_pool(name="psg", bufs=2, space="PSUM") as psg,
            tc.tile_pool(name="psx", bufs=2, space="PSUM") as psx,
            tc.tile_pool(name="dram", bufs=2, space="DRAM") as dpool,
        ):
            # ---- registers with per-core offsets (on gpsimd: dynamic DMAs live there) ----
            prm_sb = cpool.tile([1, 16], U32, tag="prm", name="prm_sb")
            nc.sync.dma_start(prm_sb[:], prm[:])
            regs = []
            with tc.tile_critical():
                for i in range(14):
                    r = nc.gpsimd.alloc_register(f"prm{i}")
                    nc.gpsimd.reg_load(r, prm_sb[0:1, i:i + 1])
                    regs.append(nc.gpsimd.snap(r, donate=True, min_val=0,
                                               max_val=8 * NH * SPAN))
            (R_OWN0, R_OWN1, R_OWNH0, R_OWNH1, R_OTH0, R_OTH1, R_OTHH0, R_OTHH1,
             R_VIT, R_FCF0, R_FCF1, R_FCB0, R_FCB1, _r13) = regs

            # ---- constants to SBUF ----
            def cload(nm, shape, dt, src):
                t = cpool.tile(shape, dt, tag=nm, name=nm)
                nc.sync.dma_start(t[:], src[:])
                return t

            xt0_sb = cload("xt0_sb", [3, L], F32, xt0)
            wih0_sb = cload("wih0_sb", [3, 1024], F32, wih0)
            fcw_sb = cload("fcw_sb", [128, 4 * NTAG], F32, fcw_in)
            fcb_sb = cload("fcb_sb", [128, NTAG], F32, fcb_in)
            trans_sb = cload("trans_sb", [128, 49], F32, trans_in)
            iot_sb = cload("iot_sb", [128, 49], F32, iot_in)
            xpadd_sb = cload("xpadd_sb", [128, 8 * W], F32, xpadd_in)
            xpsc_sb = cload("xpsc_sb", [128, 1], F32, xpsc_in)
            vmask_sb = cload("vmask_sb", [128, VSTEPS * NTAG], F32, vmask_in)
            vadd_sb = cload("vadd_sb", [128, VSTEPS * NTAG], F32, vadd_in)

            XP = xppool.tile([128, 8 * L], XP_DT, tag="xp", name="XP")

            def xp_fill(l, srcs):
                """XP[:, gc*L+pos] = (x @ WihT + b)[gc*128 + p], x = sum(srcs).

                srcs: list of (128, 4L) B-buffers (or the (3, L) xt0 for l=0).
                Chunks 0,1 (own dir) are stored in domain order; chunks 2,3
                (other dir) are stored time-reversed -> read with step -1.
                """
                if l == 0:
                    w_sb = wih0_sb
                    kcn, kstride = 1, 0
                else:
                    w_sb = wpool.tile([128, 4096], F32, tag="wih", name=f"wih_sb{l}")
                    nc.sync.dma_start(w_sb[:], wih[l][:])
                    kcn, kstride = 4, 1024
                b_sb = wpool.tile([128, 8], F32, tag="bias", name=f"bias_sb{l}")
                nc.sync.dma_start(b_sb[:], bia[l][:])
                chunks = [(i * 512, 512) for i in range(L // 512)]
                if L % 512:
                    chunks.append((L - L % 512, L % 512))
                for gc in range(8):
                    for (n0, nn) in chunks:
                        px = psx.tile([128, 512], F32, tag="px", name=f"px{l}_{gc}_{n0}")
                        nmm = kcn * len(srcs)
                        i_mm = 0
                        for src in srcs:
                            sap = src[:] if not isinstance(src, bass.AP) else src
                            for kc in range(kcn):
                                if l == 0:
                                    rhs = sap[:, n0:n0 + nn]
                                elif kc < 2:
                                    rhs = rap(sap, kc * L + n0, [sap.ap[0], [1, nn]])
                                else:
                                    rhs = rap(sap, kc * L + (L - 1 - n0),
                                              [sap.ap[0], [-1, nn]])
                                nc.tensor.matmul(
                                    px[:, 0:nn],
                                    w_sb[:, kc * kstride + gc * 128:
                                         kc * kstride + gc * 128 + 128],
                                    rhs,
                                    start=(i_mm == 0), stop=(i_mm == nmm - 1),
                                )
                                i_mm += 1
                        nc.scalar.activation(
                            XP[:, gc * L + n0: gc * L + n0 + nn], px[:, 0:nn],
                            AF.Identity, bias=b_sb[:, gc:gc + 1],
                        )
                # boundary-core pad fix on first W cols of every gc block:
                xpb = XP[:]
                hview = rap(xpb, 0, [xpb.ap[0], [L, 8], [1, W]])
                nc.vector.scalar_tensor_tensor(
                    hview, hview, xpsc_sb[0:128, 0:1],
                    xpadd_sb[:].rearrange("p (g w) -> p g w", g=8),
                    op0=ALU.mult, op1=ALU.add,
                )

            def recurrence(l, contrib_ap):
                whh_sb = wpool.tile([128, 2048], F32, tag="whh", name=f"whh_sb{l}")
                nc.sync.dma_start(whh_sb[:], whh[l][:])
                C = spool.tile([128, 2 * S], F32, tag="C", name=f"C{l}")
                H = spool.tile([128, 2 * S], F32, tag="H", name=f"H{l}")
                Hall = spool.tile([128, 2 * SPAN], F32, tag="Hall", name=f"Hall{l}")
                nc.vector.memset(C[:], 0.0)
                nc.vector.memset(H[:], 0.0)
                xpb = XP[:]
                hall = Hall[:]
                for tau in range(STEPS):
                    P = psg.tile([128, 8 * S], F32, tag="P", name=f"P{l}_{tau}")
                    for kc in range(2):
                        for gc in range(8):
                            nc.tensor.matmul(
                                P[:, gc * S:(gc + 1) * S],
                                whh_sb[:, kc * 1024 + gc * 128:
                                       kc * 1024 + gc * 128 + 128],
                                H[:, kc * S:(kc + 1) * S],
                                start=(kc == 0), stop=(kc == 1),
                            )
                    G = work.tile([128, 8 * S], F32, tag="G", name=f"G{l}_{tau}")
                    xpv = rap(xpb, tau, [xpb.ap[0], [L, 8], [LEN, S]])
                    nc.vector.tensor_tensor(
                        G[:].rearrange("p (g s) -> p g s", g=8),
                        P[:].rearrange("p (g s) -> p g s", g=8),
                        xpv, op=ALU.add)
                    SIG = work.tile([128, 6 * S], F32, tag="SIG", name=f"SIG{l}_{tau}")
                    nc.scalar.activation(SIG[:], G[:, 0:6 * S], AF.Sigmoid)
                    TG = work.tile([128, 2 * S], F32, tag="TG", name=f"TG{l}_{tau}")
                    nc.scalar.activation(TG[:], G[:, 6 * S:8 * S], AF.Tanh)
                    IG = work.tile([128, 2 * S], F32, tag="IG", name=f"IG{l}_{tau}")
                    nc.gpsimd.tensor_tensor(IG[:], SIG[:, 0:2 * S], TG[:], op=ALU.mult)
                    FCt = work.tile([128, 2 * S], F32, tag="FCt", name=f"FCt{l}_{tau}")
                    nc.vector.tensor_tensor(FCt[:], SIG[:, 2 * S:4 * S], C[:],
                                            op=ALU.mult)
                    nc.vector.tensor_tensor(C[:], IG[:], FCt[:], op=ALU.add)
                    TC = work.tile([128, 2 * S], F32, tag="TC", name=f"TC{l}_{tau}")
                    nc.scalar.activation(TC[:], C[:], AF.Tanh)
                    nc.gpsimd.tensor_tensor(H[:], SIG[:, 4 * S:6 * S], TC[:],
                                            op=ALU.mult)
                    if tau >= W:
                        nc.scalar.activation(
                            rap(hall, tau - W, [hall.ap[0], [SPAN, 2], [LEN, S]]),
                            H[:].rearrange("p (b s) -> p b s", b=2),
                            AF.Copy)
                # one contiguous archive -> HBM contribution DMA per layer
                nc.sync.dma_start(
                    rap(contrib_ap, 0, [[SPAN, 128], [128 * SPAN, 2], [1, SPAN]]),
                    hall.rearrange("p (b t) -> p b t", b=2))

            def load_half(dst_ap, src_ap, regs4):
                """8 DMAs filling a (128, 4*L) buffer from the AG output."""
                r_o0, r_o1, r_oh0, r_oh1 = regs4[0], regs4[1], regs4[2], regs4[3]
                r_t0, r_t1, r_th0, r_th1 = regs4[4], regs4[5], regs4[6], regs4[7]
                for hc, (rm, rh) in enumerate([(r_o0, r_oh0), (r_o1, r_oh1)]):
                    # own-direction: halo -> [0:W], main -> [W:L]
                    nc.gpsimd.dma_start(
                        rap(dst_ap, hc * L, [dst_ap.ap[0], [1, W]]),
                        rap(src_ap, rh, [[SPAN, 128], [1, W]]))
                    nc.gpsimd.dma_start(
                        rap(dst_ap, hc * L + W, [dst_ap.ap[0], [1, SPAN]]),
                        rap(src_ap, rm, [[SPAN, 128], [1, SPAN]]))
                for hc, (rm, rh) in enumerate([(r_t0, r_th0), (r_t1, r_th1)]):
                    # other-direction (stored reversed): main -> [0:SPAN], halo -> [SPAN:L]
                    nc.gpsimd.dma_start(
                        rap(dst_ap, (2 + hc) * L, [dst_ap.ap[0], [1, SPAN]]),
                        rap(src_ap, rm, [[SPAN, 128], [1, SPAN]]))
                    nc.gpsimd.dma_start(
                        rap(dst_ap, (2 + hc) * L + SPAN, [dst_ap.ap[0], [1, W]]),
                        rap(src_ap, rh, [[SPAN, 128], [1, W]]))

            xregs = [R_OWN0, R_OWN1, R_OWNH0, R_OWNH1, R_OTH0, R_OTH1,
                     R_OTHH0, R_OTHH1]

            # ================= main layer loop =================
            Bcur = Bprev = None
            ag_now = None
            for l in range(NLAYERS):
                if l == 0:
                    xp_fill(0, [xt0_sb])
                elif l == 1:
                    xp_fill(1, [Bcur])
                else:
                    xp_fill(l, [Bcur, Bprev])
                contrib = dpool.tile([NH, SPAN], F32, tag="contrib",
                                     name=f"contrib{l}")
                recurrence(l, contrib[:])
                if dbg_layer == l:
                    nc.sync.dma_start(dbgc_out.ap(), contrib[:])
                ag_t = dpool.tile([8 * NH, SPAN], F32, tag="ag", bufs=2,
                                  addr_space="Shared", name=f"ag{l}")
                if not os.environ.get("K_NO_CC"):
                    nc.gpsimd.collective_compute(
                        "AllGather", ALU.bypass, replica_groups=rg,
                        ins=[contrib.opt()], outs=[ag_t.opt()],
                    )
                else:
                    nc.sync.dma_start(rap(ag_t[:], 0, [[SPAN, 128], [1, SPAN]]),
                                      contrib[:])
                ag_now = ag_t[:]
                if l < NLAYERS - 1:
                    B = xpool.tile([128, 4 * L], F32, tag="xb", name=f"B{l}")
                    load_half(B[:], ag_now, xregs)
                    Bprev, Bcur = Bcur, B
                    if dbg_layer == l:
                        nc.sync.dma_start(dbgx_out.ap(), B[:])

            # ================= FC =================
            o7 = xpool.tile([128, 4 * 1024], F32, tag="xb", name="o7")
            scr = xpool.tile([128, 2 * 1024], F32, tag="xbscr", name="scr", bufs=1)
            o7a, scra = o7[:], scr[:]
            for hc, r in enumerate([R_FCF0, R_FCF1]):
                nc.gpsimd.dma_start(
                    rap(o7a, hc * 1024, [o7a.ap[0], [1, 1024]]),
                    rap(ag_now, r, [[SPAN, 128], [1, 1024]]))
            for hc, r in enumerate([R_FCB0, R_FCB1]):
                nc.gpsimd.dma_start(
                    rap(scra, hc * 1024, [scra.ap[0], [1, 1024]]),
                    rap(ag_now, r, [[SPAN, 128], [1, 1024]]))
                nc.vector.tensor_copy(
                    rap(o7a, (2 + hc) * 1024, [o7a.ap[0], [1, 1024]]),
                    rap(scra, hc * 1024 + 1023, [scra.ap[0], [-1, 1024]]))
            FT = cpool.tile([128, 8 * NTAG], F32, tag="FT", name="FT")
            for pt in range(8):
                pf = psx.tile([128, 512], F32, tag="px", name=f"pf{pt}")
                for kc in range(4):
                    nc.tensor.matmul(
                        pf[:, 0:NTAG],
                        o7[:, kc * 1024 + pt * 128: kc * 1024 + pt * 128 + 128],
                        fcw_sb[:, kc * NTAG:(kc + 1) * NTAG],
                        start=(kc == 0), stop=(kc == 3),
                    )
                nc.vector.tensor_tensor(
                    FT[:, pt * NTAG:(pt + 1) * NTAG], pf[:, 0:NTAG],
                    fcb_sb[:, 0:NTAG], op=ALU.add)
            fcontrib = dpool.tile([T // NCORE, NTAG], F32, tag="fcon",
                                  name="fcontrib")
            nc.sync.dma_start(
                rap(fcontrib[:], 0, [[NTAG, 128], [128 * NTAG, 8], [1, NTAG]]),
                FT[:].rearrange("p (t f) -> p t f", t=8))
            nc.sync.dma_start(fco_out.ap(), fcontrib[:])
            fag = dpool.tile([T, NTAG], F32, tag="fag", addr_space="Shared",
                             name="fag")
            if not os.environ.get("K_NO_CC"):
                nc.gpsimd.collective_compute(
                    "AllGather", ALU.bypass, replica_groups=rg,
                    ins=[fcontrib.opt()], outs=[fag.opt()],
                )
            else:
                nc.sync.dma_start(rap(fag[:], 0, [[NTAG, 1024], [1, NTAG]]),
                                  fcontrib[:])

            # ================= Viterbi forward (windowed) =================
            featS = cpool.tile([128, VSTEPS * NTAG], F32, tag="featS",
                               name="featS")
            nc.gpsimd.dma_start(
                featS[:],
                rap(fag[:], R_VIT, [[VLEN * NTAG, 128], [1, VSTEPS * NTAG]]))
            fS2 = cpool.tile([128, VSTEPS * NTAG], F32, tag="fS2", name="fS2")
            nc.vector.tensor_tensor(fS2[:], featS[:], vmask_sb[:], op=ALU.mult)
            nc.vector.tensor_tensor(featS[:], fS2[:], vadd_sb[:], op=ALU.add)

            fv = cpool.tile([128, NTAG], F32, tag="fv", name="fv")
            nc.vector.memset(fv[:], 0.0)
            S49 = cpool.tile([128, 49], F32, tag="S49", name="S49")
            M7 = cpool.tile([128, NTAG], F32, tag="M7", name="M7")
            EQ = cpool.tile([128, 49], F32, tag="EQ", name="EQ")
            VAL = cpool.tile([128, 49], F32, tag="VAL", name="VAL")
            BPT = cpool.tile([128, VSTEPS * NTAG], F32, tag="BPT", name="BPT")
            fva = fv[:]
            m7a = M7[:]
            for tau in range(VSTEPS):
                fvb = rap(fva, 0, [fva.ap[0], [0, NTAG], [1, NTAG]])
                nc.vector.tensor_tensor(
                    S49[:].rearrange("p (i j) -> p i j", i=NTAG),
                    trans_sb[:].rearrange("p (i j) -> p i j", i=NTAG),
                    fvb, op=ALU.add)
                nc.vector.tensor_reduce(
                    M7[:], S49[:].rearrange("p (i j) -> p i j", i=NTAG),
                    axis=mybir.AxisListType.X, op=ALU.max)
                m7b = rap(m7a, 0, [m7a.ap[0], [1, NTAG], [0, NTAG]])
                nc.vector.tensor_tensor(
                    EQ[:].rearrange("p (i j) -> p i j", i=NTAG),
                    S49[:].rearrange("p (i j) -> p i j", i=NTAG),
                    m7b, op=ALU.is_equal)
                nc.vector.scalar_tensor_tensor(
                    VAL[:], EQ[:], -BIG, iot_sb[:], op0=ALU.mult, op1=ALU.add)
                nc.vector.tensor_reduce(
                    BPT[:, tau * NTAG:(tau + 1) * NTAG],
                    VAL[:].rearrange("p (i j) -> p i j", i=NTAG),
                    axis=mybir.AxisListType.X, op=ALU.min)
                nc.vector.tensor_tensor(
                    fv[:], M7[:], featS[:, tau * NTAG:(tau + 1) * NTAG],
                    op=ALU.add)
            nc.sync.dma_start(bpt_out.ap(), BPT[:, VW * NTAG:])
            nc.sync.dma_start(fvo_out.ap(), fv[127:128, :])

    nc.compile()
    return nc


# ---------------------------------------------------------------------------
# host-side input prep
# ---------------------------------------------------------------------------

def _weights_T(w_perm):
    """(1024, D) gate-permuted weight -> lhsT layout (128, (D/128)*1024)."""
    D = w_perm.shape[1]
    kcn = D // 128
    out = np.empty((128, kcn * 1024), np.float32)
    for kc in range(kcn):
        for gc in range(8):
            out[:, kc * 1024 + gc * 128: kc * 1024 + (gc + 1) * 128] = \
                w_perm[gc * 128:(gc + 1) * 128, kc * 128:(kc + 1) * 128].T
    return np.ascontiguousarray(out)


def _wih0_T(wih0_perm):
    """(1024, 3) -> (3, 1024): [gc*128+j] cols, K=3 partitions."""
    out = np.empty((3, 1024), np.float32)
    for gc in range(8):
        out[:, gc * 128:(gc + 1) * 128] = wih0_perm[gc * 128:(gc + 1) * 128, :].T
    return out


def make_inputs(sentence, lstm_params, fc_w, fc_b, transitions):
    sentence = np.asarray(sentence, np.float32)
    fc_w = np.asarray(fc_w, np.float32)
    fc_b = np.asarray(fc_b, np.float32)
    transitions = np.asarray(transitions, np.float32)

    sentT = np.ascontiguousarray(sentence.T)              # (3, T)
    sentT_rev = np.ascontiguousarray(sentence[::-1].T)    # (3, T) reversed

    in_maps = []
    for c in range(NCORE):
        d = c // 4          # 0 fwd, 1 bwd
        k = c % 4
        m = {}
        # --- layer-0 x (3, L), domain [k*SPAN-W, (k+1)*SPAN) in processing order
        st = sentT if d == 0 else sentT_rev
        x0 = np.zeros((3, L), np.float32)
        lo = k * SPAN - W
        src_lo = max(0, lo)
        x0[:, src_lo - lo:] = st[:, src_lo:(k + 1) * SPAN]
        m["xt0"] = x0
        # --- weights
        for l in range(NLAYERS):
            p = lstm_params[l]
            wih_ = np.asarray(p["fWih" if d == 0 else "rWih"], np.float32)
            whh_ = np.asarray(p["fWhh" if d == 0 else "rWhh"], np.float32)
            b_ = np.asarray(p["fb" if d == 0 else "rb"], np.float32)
            wih_p = wih_[GPERM]
            whh_p = whh_[GPERM]
            b_p = b_[GPERM]
            if l >= 1 and d == 1:
                # bwd cores see features ordered [own(bwd), other(fwd)]
                wih_p = np.concatenate([wih_p[:, 256:], wih_p[:, :256]], axis=1)
            if l == 0:
                m["wih0"] = _wih0_T(wih_p)
            else:
                m[f"wih{l}"] = _weights_T(wih_p)
            m[f"whh{l}"] = _weights_T(whh_p)
            bb = np.empty((128, 8), np.float32)
            for gc in range(8):
                bb[:, gc] = b_p[gc * 128:(gc + 1) * 128]
            m[f"bias{l}"] = bb
        # --- fc
        fcw = np.empty((128, 4 * NTAG), np.float32)
        for kc in range(4):
            fcw[:, kc * NTAG:(kc + 1) * NTAG] = fc_w[:, kc * 128:(kc + 1) * 128].T
        m["fcw"] = fcw
        m["fcb"] = np.tile(fc_b[None, :], (128, 1)).astype(np.float32)
        # --- viterbi constants
        m["transB"] = np.tile(transitions.reshape(1, 49), (128, 1)).astype(np.float32)
        m["iotB"] = np.tile(
            np.tile(np.arange(NTAG, dtype=np.float32) + BIG, NTAG)[None, :],
            (128, 1)).astype(np.float32)
        # --- xp pad fix
        boundary = (k == 0)
        xpadd = np.zeros((128, 8 * W), np.float32)
        if boundary:
            for gc in (0, 1):   # i-gate chunks
                xpadd[:, gc * W:(gc + 1) * W] = NEG
        m["xpadd"] = xpadd
        m["xpsc"] = np.full((128, 1), 0.0 if boundary else 1.0, np.float32)
        # --- viterbi pad fix + featS offset
        vmask = np.ones((128, VSTEPS * NTAG), np.float32)
        vadd = np.zeros((128, VSTEPS * NTAG), np.float32)
        vit_row0 = c * (T // NCORE) - VW
        if c == 0:
            pad = np.full(NTAG, NEG, np.float32)
            pad[START_TAG] = -NEG
            for w_ in range(VWIN):
                for tau_ in range(VSTEPS):
                    pos = vit_row0 + w_ * VLEN + tau_
                    if pos < 0:
                        sl = slice(tau_ * NTAG, (tau_ + 1) * NTAG)
                        vmask[w_, sl] = 0.0
                        vadd[w_, sl] = pad
        m["vmask"] = vmask
        m["vadd"] = vadd
        # --- prm offsets (elements)
        oth_main_blk = 4 * (1 - d) + (3 - k)
        oth_halo_blk = 4 * (1 - d) + (4 - k) if k != 0 else 4 * (1 - d)
        own_halo_blk = c - 1 if k != 0 else c
        pr = np.zeros(16, np.uint32)
        pr[0] = (c * NH + 0) * SPAN
        pr[1] = (c * NH + 128) * SPAN
        pr[2] = (own_halo_blk * NH + 0) * SPAN + (SPAN - W)
        pr[3] = (own_halo_blk * NH + 128) * SPAN + (SPAN - W)
        pr[4] = (oth_main_blk * NH + 0) * SPAN
        pr[5] = (oth_main_blk * NH + 128) * SPAN
        pr[6] = (oth_halo_blk * NH + 0) * SPAN
        pr[7] = (oth_halo_blk * NH + 128) * SPAN
        pr[8] = max(0, vit_row0) * NTAG
        fwd_blk = c // 2
        fwd_col = 1024 * (c % 2)
        bwd_blk = 4 + (7 - c) // 2
        bwd_col = (8191 - 1024 * c - 2048 * ((7 - c) // 2)) - 1023
        pr[9] = (fwd_blk * NH + 0) * SPAN + fwd_col
        pr[10] = (fwd_blk * NH + 128) * SPAN + fwd_col
        pr[11] = (bwd_blk * NH + 0) * SPAN + bwd_col
        pr[12] = (bwd_blk * NH + 128) * SPAN + bwd_col
        m["prm"] = pr.reshape(1, 16)
        in_maps.append(m)
    return in_maps


# ---------------------------------------------------------------------------
# host-side post-processing
# ---------------------------------------------------------------------------

def postprocess(results, transitions):
    transitions = np.asarray(transitions, np.float32)
    feats = np.concatenate([results[c]["fco"] for c in range(NCORE)], axis=0)
    bp = np.empty((T, NTAG), np.int64)
    for c in range(NCORE):
        b = results[c]["bpt"].reshape(VWIN * VLEN, NTAG)
        bp[c * (T // NCORE):(c + 1) * (T // NCORE)] = b.astype(np.int64)
    fvT = results[7]["fvo"][0]
    term = fvT + transitions[STOP_TAG]
    best = int(np.argmax(term))

    # pointer-doubling backtrace:
    # M_t = bp[t] maps tag@t -> tag@(t-1);  path[t] = (M_{t+1} o ... o M_{T-1})(best)
    # B^k[t] = M_t o M_{t+1} o ... o M_{t+2^k-1}  (rightmost applied first)
    B = bp.copy()
    idx = np.arange(T)
    step = 1
    while step < T:
        shifted = np.minimum(idx + step, T - 1)
        valid = idx + step <= T - 1
        gathered = np.take_along_axis(B, np.where(valid[:, None],
                                                  B[shifted], 0), axis=1)
        B = np.where(valid[:, None], gathered, B)
        step *= 2
    path = np.empty(T, np.int64)
    path[T - 1] = best
    path[:T - 1] = B[1:, best]

    prev = np.concatenate([[START_TAG], path[:-1]])
    score = (feats[np.arange(T), path].astype(np.float64).sum()
             + transitions[path, prev].astype(np.float64).sum()
             + float(transitions[STOP_TAG, path[-1]]))
    return np.float32(score), path.astype(np.int32), feats


# ---------------------------------------------------------------------------
# entry point
# ---------------------------------------------------------------------------

def kernel(sentence, lstm_params, fc_w, fc_b, transitions, _res_out=None):
    in_maps = make_inputs(sentence, lstm_params, fc_w, fc_b, transitions)
    nc = build_program()
    trace = bool(os.environ.get("K_TRACE"))
    res = run_bass_kernel_spmd(nc, in_maps, core_ids=list(range(NCORE)),
                               trace=trace)
    if _res_out is not None:
        _res_out.append(res)
    score, path, _ = postprocess(res.results, transitions)
    return score, path
